# revision 1
# baseline (speedup 1.0000x reference)
"""Causal single-head attention (S=4096, dmodel=1024, dk=128) on 8 TRN2 cores.

Launch 1 (proj): core c computes K^T/V^T/Q^T for rows c::8 from ONE X slice
load (vs two in the older design), chunk-streamed DMA overlapped with the
contraction matmuls; the three output copies run in parallel on Act/DVE/Act
(separate PSUM tiles per projection -- Tile tracks deps at tile granularity).

Launch 2 (attn): interleaved queries c::8 vs full K/V.  16-query causal
granularity (512-16m valid window per key chunk, ~18% less work than 128-q
granularity); exp batched into 13 activation tiles; each key chunk gets a
bank-aligned 512-f32 PSUM slot (HW corrupts matmul writes crossing a 2KB PSUM
bank boundary) with exactly one start=True write per bank; causal masking is
additive on the PE (head/diagonal get -100 via I100 matmuls, exp underflows
to 0) so the vector engine never touches masks; denominator via a DVE bf16
add-tree + one ones-matmul per tile (replacing the per-chunk PE sum matmuls);
the work(i) stage (AV + denominator) is delayed one pipeline step behind
scores(i+1)/exp(i) so the PE never stalls on the activation ack.
"""

import math

import numpy as np
import ml_dtypes

S = 4096
D = 1024
DK = 128
NCORES = 8
SL = S // NCORES          # 512 queries per core
P = 128
NKC = S // P              # 32 key chunks
DCH = D // P              # 8 contraction chunks

BF16 = ml_dtypes.bfloat16

# attn exp-tile packing: (m0, n) -> chunks m0..m0+n-1 at width W = 512-16*m0.
# Each chunk gets a bank-aligned 512-f32 PSUM slot (HW corrupts matmul writes
# that cross a 2KB PSUM bank boundary), so n <= 3 per 3-bank pscore buffer.
TILES = [(0, 1), (1, 3), (4, 3), (7, 3), (10, 3), (13, 3), (16, 3), (19, 3),
         (22, 3), (25, 3), (28, 3), (31, 1)]
KVGROUPS = [[0, 1], [2, 3], [4, 5, 6], [7, 8, 9], [10, 11]]
KVOFF = []
_o = 0
for _m0, _n in TILES:
    KVOFF.append(_o)
    _o += 2 * _n * P
KVLEN = _o  # 8192
OUT_EARLY = 24 * 16  # outT/den cols [0, 384) final after tile (18,6)

_CACHE = {}


def _build_proj():
    import concourse.mybir as mybir
    from concourse import bacc
    from contextlib import ExitStack
    from concourse.tile import TileContext

    f32 = mybir.dt.float32
    bf16 = mybir.dt.bfloat16
    COPY = mybir.ActivationFunctionType.Copy

    CW = SL + 3 * DK          # packed row: x | wk | wv | wq
    nc = bacc.Bacc(None, name="proj")
    allin = nc.dram_tensor("allin", [D, CW], bf16, kind="ExternalInput")
    biasT = nc.dram_tensor("biasT", [DK, 3], f32, kind="ExternalInput")
    kts = nc.dram_tensor("kts", [P, SL], bf16, kind="ExternalOutput")
    vss = nc.dram_tensor("vss", [P, SL], bf16, kind="ExternalOutput")
    qts = nc.dram_tensor("qts", [P, SL], bf16, kind="ExternalOutput")

    with TileContext(nc) as tc, ExitStack() as ctx:
        pool = ctx.enter_context(tc.tile_pool(name="pool", bufs=1))
        psum = ctx.enter_context(tc.tile_pool(name="psum", bufs=1, space="PSUM"))

        ain = pool.tile([P, DCH, CW], bf16)
        ain_r = allin[:, :].rearrange("(c p) w -> p c w", p=P)
        for d in range(DCH):
            nc.sync.dma_start(ain[:, d], ain_r[:, d])
        bT = pool.tile([DK, 3], f32)
        nc.sync.dma_start(bT, biasT[:, :])

        ones_w = pool.tile([P, 8], bf16)
        nc.vector.memset(ones_w, 1.0)

        # separate PSUM tiles: Tile tracks deps per-tile, so distinct tiles
        # let the three output copies run on different engines in parallel
        psP = [psum.tile([P, SL], f32, name=f"psP{t}") for t in range(3)]

        # wide warm-ups: keep the PE p-state streak alive until data arrives
        for _ in range(190):
            nc.tensor.matmul(psP[0][0:1, 0:8], lhsT=ones_w[:, 0:1], rhs=ones_w,
                             start=True, stop=True, skip_group_check=True)

        xs = ain[:, :, 0:SL]
        for d in range(DCH - 1):
            for t in range(3):
                w = ain[:, d, SL + DK * t:SL + DK * (t + 1)]
                nc.tensor.matmul(psP[t], lhsT=w, rhs=xs[:, d],
                                 start=(d == 0), stop=False,
                                 skip_group_check=True)
        # last chunk per projection with stop, V first: its copy + DMA chain
        # runs on the (promptly-woken) DVE while K/Q matmuls still execute;
        # Act handles only Q (its first post-idle instruction wakes ~0.8us
        # late, gated on the batched PE semaphore)
        d = DCH - 1
        for t in (1, 0, 2):
            w = ain[:, d, SL + DK * t:SL + DK * (t + 1)]
            nc.tensor.matmul(psP[t], lhsT=w, rhs=xs[:, d],
                             start=False, stop=True, skip_group_check=True)

        IDENT = mybir.ActivationFunctionType.Identity
        k_sb = pool.tile([P, SL], bf16)
        v_sb = pool.tile([P, SL], bf16)
        q_sb = pool.tile([P, SL], bf16)
        nc.vector.tensor_scalar_add(v_sb, psP[1], bT[:, 1:2])
        nc.sync.dma_start(vss[:, :], v_sb)
        nc.vector.tensor_scalar_add(k_sb, psP[0], bT[:, 0:1])
        nc.gpsimd.dma_start(kts[:, :], k_sb)
        nc.scalar.activation(q_sb, psP[2], IDENT, bias=bT[:, 2:3])
        nc.scalar.dma_start(qts[:, :], q_sb)

    nc.finalize()
    return nc


def _build_attn():
    import concourse.mybir as mybir
    from concourse import bacc
    from contextlib import ExitStack
    from concourse.tile import TileContext

    f32 = mybir.dt.float32
    bf16 = mybir.dt.bfloat16
    EXP = mybir.ActivationFunctionType.Exp
    ADD = mybir.AluOpType.add

    nc = bacc.Bacc(None, name="attn")
    # qti = qT (512) | negtri (16) | i100 (128) | kv block of tile 0
    KV0 = 2 * TILES[0][1] * P
    qti = nc.dram_tensor("qti", [P, SL + 16 + P + KV0], bf16,
                         kind="ExternalInput")
    kv = nc.dram_tensor("kv", [P, KVLEN], bf16, kind="ExternalInput")
    outT = nc.dram_tensor("outT", [DK, SL], bf16, kind="ExternalOutput")
    den = nc.dram_tensor("den", [1, SL], f32, kind="ExternalOutput")

    with TileContext(nc) as tc, ExitStack() as ctx:
        pool = ctx.enter_context(tc.tile_pool(name="pool", bufs=1))
        epool = ctx.enter_context(tc.tile_pool(name="epool", bufs=3))
        spool = ctx.enter_context(tc.tile_pool(name="spool", bufs=6))
        pscore = ctx.enter_context(tc.tile_pool(name="pscore", bufs=2, space="PSUM"))
        pacc = ctx.enter_context(tc.tile_pool(name="pacc", bufs=1, space="PSUM"))

        qti_sb = pool.tile([P, SL + 16 + P + KV0], bf16)
        qT = qti_sb[:, 0:SL]
        tri_sb = qti_sb[:, SL:SL + 16]
        i100_sb = qti_sb[:, SL + 16:SL + 16 + P]
        kv0_sb = qti_sb[:, SL + 16 + P:]
        kv_sb = pool.tile([P, KVLEN], bf16)
        nc.sync.dma_start(qti_sb, qti[:, :])
        for grp in KVGROUPS:
            if grp[0] == 0:
                grp = grp[1:]
            if not grp:
                continue
            lo = KVOFF[grp[0]]
            hi = KVOFF[grp[-1]] + 2 * TILES[grp[-1]][1] * P
            nc.sync.dma_start(kv_sb[:, lo:hi], kv[:, lo:hi])

        def ksl(i, j):  # k chunk j of tile i
            if i == 0:
                return kv0_sb[:, P * j:P * (j + 1)]
            return kv_sb[:, KVOFF[i] + P * j:KVOFF[i] + P * (j + 1)]

        def vsl(i, j):  # v chunk j of tile i
            n = TILES[i][1]
            if i == 0:
                return kv0_sb[:, P * (n + j):P * (n + j + 1)]
            return kv_sb[:, KVOFF[i] + P * (n + j):KVOFF[i] + P * (n + j + 1)]

        ones_col = pool.tile([P, 1], bf16)
        nc.vector.memset(ones_col, 1.0)
        negones = pool.tile([P, 112], bf16)
        nc.vector.memset(negones, -1.0)

        psAV = pacc.tile([DK, SL], f32, tag="av")
        psSum = pacc.tile([1, SL], f32, tag="sum")
        ob = pool.tile([DK, SL], bf16)
        db = pool.tile([1, SL], f32)

        for _ in range(290):
            nc.tensor.matmul(psSum[0:1, 0:8], lhsT=ones_col, rhs=negones[:, 0:8],
                             start=True, stop=True, skip_group_check=True)

        ntile = len(TILES)

        def scores(i):
            m0, n = TILES[i]
            W = SL - 16 * m0
            # one bank (512 f32) per chunk slot; only [0:W) is ever read.
            # Exactly one start=True write per bank (the scores matmul);
            # head/diag accumulate -100 on top (stale data there is bounded,
            # so exp still underflows to zero).
            psS = pscore.tile([P, n, 512], f32, tag="ps")
            for j in range(n):
                m = m0 + j
                off = 16 * j
                nc.tensor.matmul(psS[:, j, off:W], lhsT=ksl(i, j),
                                 rhs=qT[:, 16 * m:], start=True, stop=False,
                                 skip_group_check=True)
                if off:
                    nc.tensor.matmul(psS[:, j, 0:off], lhsT=i100_sb,
                                     rhs=negones[:, 0:off],
                                     start=False, stop=False,
                                     skip_group_check=True)
                nc.tensor.matmul(psS[:, j, off:off + 16], lhsT=i100_sb,
                                 rhs=tri_sb, start=False, stop=True,
                                 skip_group_check=True)
            return psS

        def work(i, e):
            """AV + denominator + streamed output for exp'd tile i."""
            m0, n = TILES[i]
            W = SL - 16 * m0
            for j in range(n):
                m = m0 + j
                nc.tensor.matmul(psAV[:, 16 * m:], lhsT=vsl(i, j),
                                 rhs=e[:, j, 16 * j:],
                                 start=(m == 0), stop=(m == NKC - 1),
                                 skip_group_check=True)
            # binary-tree key-chunk reduction on DVE, final sums to PE
            level = [e[:, j] for j in range(n)]
            tag = 0
            while len(level) > 1:
                nxt = []
                for j0 in range(0, len(level) - 1, 2):
                    t = spool.tile([P, W], bf16, tag=f"s{tag}")
                    nc.vector.tensor_tensor(t, level[j0], level[j0 + 1], ADD)
                    nxt.append(t)
                    tag += 1
                if len(level) % 2:
                    nxt.append(level[-1])
                level = nxt
            for qi, qs in enumerate(level):
                nc.tensor.matmul(psSum[:, 16 * m0:], lhsT=ones_col, rhs=qs,
                                 start=(i == 0 and qi == 0),
                                 stop=(i == ntile - 1 and qi == len(level) - 1),
                                 skip_group_check=True)

        # work(i) is delayed one iteration so the AV matmuls consume an
        # exp tile whose ack has long fired -- PE never stalls on Act
        psS = scores(0)
        prev = None
        for i in range(ntile):
            e = epool.tile([P, TILES[i][1], SL - 16 * TILES[i][0]], bf16, tag="e")
            nc.scalar.activation(e, psS[:, :, 0:SL - 16 * TILES[i][0]], EXP)
            if i + 1 < ntile:
                psS = scores(i + 1)
            if prev is not None:
                work(i - 1, prev)
            prev = e
        work(ntile - 1, prev)

        nc.scalar.activation(ob, psAV, mybir.ActivationFunctionType.Copy)
        nc.sync.dma_start(outT[:, :], ob)
        nc.vector.tensor_copy(db, psSum)
        nc.scalar.dma_start(den[:, :], db)

    nc.finalize()
    return nc


def get_ncs():
    if "nc1" not in _CACHE:
        _CACHE["nc1"] = _build_proj()
        _CACHE["nc2"] = _build_attn()
    return _CACHE["nc1"], _CACHE["nc2"]


def make_in_maps1(inputs):
    X = np.asarray(inputs["X"], np.float32)
    Wq = np.asarray(inputs["Wq"], np.float32)
    bq = np.asarray(inputs["bq"], np.float32)
    Wk = np.asarray(inputs["Wk"], np.float32)
    bk = np.asarray(inputs["bk"], np.float32)
    Wv = np.asarray(inputs["Wv"], np.float32)
    bv = np.asarray(inputs["bv"], np.float32)
    scale = 1.0 / math.sqrt(DK)
    WkT16 = np.ascontiguousarray(Wk.T).astype(BF16)
    WvT16 = np.ascontiguousarray(Wv.T).astype(BF16)
    WqT16 = np.ascontiguousarray((Wq * scale).T).astype(BF16)
    biasT = np.stack([bk, bv, bq * scale], axis=1).astype(np.float32)  # [DK,3]
    maps = []
    for c in range(NCORES):
        xct = np.ascontiguousarray(X[c::NCORES].T).astype(BF16)  # [1024, 512]
        allin = np.ascontiguousarray(
            np.concatenate([xct, WkT16, WvT16, WqT16], axis=1))
        maps.append({"allin": allin, "biasT": biasT})
    return maps


def make_in_maps2(res1):
    # K^T/V^T/Q^T tiles hold rows c'::8: core c' col j <-> row/key 8j+c'
    karr = np.stack([np.asarray(r["kts"]) for r in res1], axis=0)  # [8,128,512]
    ktf = karr.transpose(1, 2, 0).reshape(P, S)                    # K^T [dk, 4096]
    varr = np.stack([np.asarray(r["vss"]) for r in res1], axis=0)
    vtf = varr.transpose(1, 2, 0).reshape(P, S)                    # V^T [dv, 4096]
    i100 = (np.eye(P, dtype=np.float32) * 100.0).astype(BF16)
    kr = np.arange(P)[:, None]          # key lane
    tt = np.arange(16)[None, :]         # diag col
    maps = []
    for c in range(NCORES):
        negtri = np.where(8 * tt + c >= kr, 0.0, -1.0).astype(BF16)
        kvparts = []
        for (m0, n) in TILES:
            kvparts.append(ktf[:, P * m0:P * (m0 + n)])
            kvparts.append(np.concatenate(
                [vtf[:, P * m:P * (m + 1)].T for m in range(m0, m0 + n)], axis=1))
        kvf = np.ascontiguousarray(np.concatenate(kvparts, axis=1))
        kv0 = kvf[:, 0:2 * TILES[0][1] * P]
        qti = np.ascontiguousarray(
            np.concatenate([np.asarray(res1[c]["qts"]), negtri, i100, kv0],
                           axis=1))
        maps.append({"qti": qti, "kv": kvf})
    return maps


LAST_RESULTS = None


def kernel(**inputs) -> np.ndarray:
    global LAST_RESULTS
    from concourse.bass_utils import run_bass_kernel_spmd

    nc1, nc2 = get_ncs()
    res1 = run_bass_kernel_spmd(nc1, make_in_maps1(inputs),
                                core_ids=list(range(NCORES)))
    res2 = run_bass_kernel_spmd(nc2, make_in_maps2(res1.results),
                                core_ids=list(range(NCORES)))
    LAST_RESULTS = (res1, res2)
    out = np.empty((S, DK), np.float32)
    for c in range(NCORES):
        ctxT = np.asarray(res2.results[c]["outT"], np.float32)
        dnm = np.asarray(res2.results[c]["den"], np.float32)
        out[c::NCORES] = (ctxT / dnm).T
    return out

